# revision 2
# baseline (speedup 1.0000x reference)
"""Multi-head attention Trainium2 kernel, v2.

Problem: B=2, S=4096, D=512, H=8 heads, dk=dv=64 (fp32 in/out).
Sharding: head-parallel - core c computes head c for both batches.

Key structural changes vs v1:
  - x is pre-cast to bf16 on the host and loaded TRANSPOSED via the DMA
    xbar (dma_start(transpose=True)); no PE transposes / PSUM round-trip
    for x^T at all.
  - Q^T and K^T are produced by one packed projection per token group
    (stationary [Wq|Wk] chunk [128,128]); psum rows 0:64 = Q^T,
    64:128 = K^T, drained (+bias) by DVE (Q) and ACT (K) into qkt fp32r.
  - V is computed in NATURAL layout (keys on partitions) via
    stationary-swap (x^T chunks stationary, Wv moving), +bv folded in
    during the psum drain; stored bf16 with a ones-column (denominator
    trick).
  - Scores: S^T = K^T.T @ Q^T row-packed across batches (tile_position
    (0,0)/(64,0)) into ONE [128,1024] psum pair tile; fp32r operands.
  - exp is split between ACT (exact, activation Exp) and DVE
    (Schraudolph bit-trick: bf16 = bitcast(int16(s*A+B)), ~3% per-elem
    err) on a per-key-block static schedule; output P^T in bf16.
  - PV uses the stationary-swap form: stationary = P^T block [128,128]
    bf16 (FWL), moving = [V|1] bf16 [128,65], accumulated over key
    blocks -> output lands in NATURAL layout [queries, 64+denom].
    No epilogue transposes; normalize = DVE reciprocal + per-partition
    scale on DVE/ACT, then direct DMA out.
"""

import sys

sys.path.insert(0, "/opt/trn_rl_repo")

import numpy as np
import ml_dtypes

import concourse.bass as bass
import concourse.tile as tile
from concourse import bacc, mybir
from concourse.bass_utils import run_bass_kernel_spmd

FP32 = mybir.dt.float32
FP32R = mybir.dt.float32r
BF16 = mybir.dt.bfloat16
I16 = mybir.dt.int16

B = 2
S = 4096
D = 512
DK = 64
HEADS = 8
N_CORES = 8

TG = 512          # tokens per phase-A group
QG = 512          # queries per phase-B group
KB = 128          # keys per block

# Schraudolph exp: bf16 bits = int16(round(s * EXP_A + EXP_B))
# EXP_A = 0.125 * 128 * log2(e); EXP_B = 127*128 - 5.5 (max-err balanced)
EXP_A = 23.083120654223414
EXP_B = 16250.5

# key blocks (per qg) whose exp runs on DVE via Schraudolph; rest on ACT
DVE_KB = tuple(range(1, 32, 4))
PV_LAG_CFG = 2


def build_nc(s=S, reps=1, phases="AB", dve_kb=DVE_KB):
    toks = B * s
    n_tg = toks // TG
    n_qg = s // QG
    n_kb = s // KB

    nc = bacc.Bacc("TRN2", target_bir_lowering=False, debug=False,
                   num_devices=N_CORES)

    xbf_d = nc.dram_tensor("xbf", [4, toks, 128], BF16, kind="ExternalInput")
    wqk_d = nc.dram_tensor("wqk", [D, 128], BF16, kind="ExternalInput")
    wv_d = nc.dram_tensor("wv", [D, DK], BF16, kind="ExternalInput")
    bq_d = nc.dram_tensor("bq", [DK, 1], FP32, kind="ExternalInput")
    bk_d = nc.dram_tensor("bk", [DK, 1], FP32, kind="ExternalInput")
    bv4_d = nc.dram_tensor("bv4", [128, 256], FP32, kind="ExternalInput")
    out_d = nc.dram_tensor("out", [toks, DK], FP32, kind="ExternalOutput")

    with tile.TileContext(nc) as tc:
        with tc.tile_pool(name="persist", bufs=1) as pp:
            wqk_sb = pp.tile([128, 4, 128], BF16, tag="wqk")
            nc.sync.dma_start(wqk_sb[:],
                              wqk_d.rearrange("(c p) m -> p c m", p=128))
            wv_sb = pp.tile([128, 4, DK], BF16, tag="wv")
            nc.sync.dma_start(wv_sb[:],
                              wv_d.rearrange("(c p) m -> p c m", p=128))
            bq_t = pp.tile([DK, 1], FP32, tag="bq")
            bk_t = pp.tile([DK, 1], FP32, tag="bk")
            nc.sync.dma_start(bq_t[:], bq_d[:])
            nc.sync.dma_start(bk_t[:], bk_d[:])
            bv4_t = pp.tile([128, 256], FP32, tag="bv4")
            nc.sync.dma_start(bv4_t[:], bv4_d[:])

            # Q^T | K^T, both batches: rows 0:64 = batch 0, 64:128 = b1;
            # free: [0, s) = Q^T, [s, 2s) = K^T.  fp32r for exact scores.
            qkt = pp.tile([128, 2 * s], FP32R, tag="qkt")
            # V natural (keys on partitions), bf16, per (batch, key block)
            # 65 cols: 0:64 = V+bv, 64 = 1.0 (denominator column).
            v_sb = pp.tile([128, B * n_kb * 65], BF16, tag="v_sb")
            v3 = v_sb[:].rearrange("p (g c) -> p g c", c=65)
            nc.vector.memset(v3[:, :, 64:65], 1.0)

            for _rep in range(reps):
                _phases(nc, tc, s, n_tg, n_qg, n_kb, xbf_d, out_d,
                        wqk_sb, wv_sb, bq_t, bk_t, bv4_t, qkt, v3,
                        phases, dve_kb)
    nc.compile()
    return nc


def _phases(nc, tc, s, n_tg, n_qg, n_kb, xbf_d, out_d,
            wqk_sb, wv_sb, bq_t, bk_t, bv4_t, qkt, v3, phases, dve_kb):
    # ---------------- Phase A ----------------
    with tc.tile_pool(name="pha_xt", bufs=3) as pxt, \
         tc.tile_pool(name="pha_qk_ps", bufs=2, space="PSUM") as pqk, \
         tc.tile_pool(name="pha_v_ps", bufs=2, space="PSUM") as pvs:
        for tg in range(n_tg):
            b = tg // (n_tg // B)
            toff = (tg % (n_tg // B)) * TG
            rb = 64 * b
            t0 = b * s + toff
            xt = pxt.tile([128, 4, TG], BF16, tag="xt")
            for c in range(4):
                nc.sync.dma_start(xt[:, c, :], xbf_d[c, t0:t0 + TG, :],
                                  transpose=True)
            psqk = pqk.tile([128, TG], FP32, tag="psqk")
            for c in range(4):
                nc.tensor.matmul(psqk[:], wqk_sb[:, c, :], xt[:, c, :],
                                 start=(c == 0), stop=(c == 3))
            nc.vector.tensor_scalar(
                out=qkt[rb:rb + 64, toff:toff + TG], in0=psqk[0:64, :],
                scalar1=bq_t[:], scalar2=None, op0=mybir.AluOpType.add)
            nc.scalar.activation(
                qkt[rb:rb + 64, s + toff:s + toff + TG], psqk[64:128, :],
                mybir.ActivationFunctionType.Identity, bias=bk_t[:])
            vps = pvs.tile([128, 4 * DK], FP32, tag="vps")
            for a in range(4):
                for c in range(4):
                    nc.tensor.matmul(
                        vps[:, a * DK:(a + 1) * DK],
                        xt[:, c, a * 128:(a + 1) * 128],
                        wv_sb[:, c, :], start=(c == 0), stop=(c == 3))
            g0 = b * n_kb + (toff // 128)
            nc.vector.tensor_tensor(
                out=v3[:, g0:g0 + 4, 0:64],
                in0=vps[:].rearrange("p (a m) -> p a m", m=DK),
                in1=bv4_t[:].rearrange("p (a m) -> p a m", m=DK),
                op=mybir.AluOpType.add)

    # ---------------- Phase B ----------------
    if "B" not in phases:
        return
    from contextlib import ExitStack
    with ExitStack() as stack:
        pscA = stack.enter_context(tc.tile_pool(
            name="phb_scA", bufs=(2 if dve_kb else 3), space="PSUM"))
        pscD = (stack.enter_context(tc.tile_pool(
            name="phb_scD", bufs=1, space="PSUM")) if dve_kb else pscA)
        pop = stack.enter_context(tc.tile_pool(
            name="phb_pv", bufs=2, space="PSUM"))
        pexpA = stack.enter_context(tc.tile_pool(name="phb_expA", bufs=PV_LAG_CFG + 2))
        pexpD = stack.enter_context(tc.tile_pool(name="phb_expD", bufs=2))
        paug = stack.enter_context(tc.tile_pool(name="phb_aug",
                                                bufs=2 * n_qg))
        pout = stack.enter_context(tc.tile_pool(name="phb_out", bufs=4))
        prcp = stack.enter_context(tc.tile_pool(name="phb_rcp", bufs=4))
        augs = []  # (qg, b, aug) stash; normalized in phase C
        PV_LAG = PV_LAG_CFG  # software-pipeline: PV lags scores
        for qg in range(n_qg):
            opsA = pop.tile([128, 4 * 65], FP32, tag="ops")
            opsB = pop.tile([128, 4 * 65], FP32, tag="ops")
            pes = {}
            for kbi in range(n_kb + PV_LAG):
                if kbi < n_kb:
                    kb = kbi
                    dve = kb in dve_kb
                    ps = (pscD if dve else pscA).tile([128, 2 * QG], FP32,
                                                      tag="sc")
                    nc.tensor.matmul(
                        ps[:, 0:QG],
                        qkt[0:64, s + kb * 128:s + (kb + 1) * 128],
                        qkt[0:64, qg * QG:(qg + 1) * QG],
                        start=True, stop=True, tile_position=(0, 0))
                    nc.tensor.matmul(
                        ps[:, QG:2 * QG],
                        qkt[64:128, s + kb * 128:s + (kb + 1) * 128],
                        qkt[64:128, qg * QG:(qg + 1) * QG],
                        start=True, stop=True, tile_position=(64, 0))
                    pe_t = (pexpD if dve else pexpA).tile(
                        [128, 2 * QG], BF16, tag="pexp")
                    if dve:
                        nc.vector.tensor_scalar(
                            out=pe_t[:].bitcast(I16), in0=ps[:],
                            scalar1=EXP_A, scalar2=EXP_B,
                            op0=mybir.AluOpType.mult,
                            op1=mybir.AluOpType.add)
                    else:
                        nc.scalar.activation(
                            pe_t[:], ps[:],
                            mybir.ActivationFunctionType.Exp, scale=0.125)
                    pes[kb] = pe_t
                if kbi < PV_LAG:
                    continue
                kb = kbi - PV_LAG
                pe_t = pes.pop(kb)
                for c in range(4):
                    # start=True clears has_written for the whole PSUM bank,
                    # so only the tile's FIRST matmul may set it; the other
                    # regions rely on has_written=0 -> overwrite semantics.
                    nc.tensor.matmul(
                        opsA[:, c * 65:(c + 1) * 65],
                        pe_t[:, c * 128:(c + 1) * 128],
                        v3[:, kb, :],
                        start=(kb == 0 and c == 0), stop=(kb == n_kb - 1),
                        skip_group_check=True)
                    nc.tensor.matmul(
                        opsB[:, c * 65:(c + 1) * 65],
                        pe_t[:, QG + c * 128:QG + (c + 1) * 128],
                        v3[:, n_kb + kb, :],
                        start=(kb == 0 and c == 0), stop=(kb == n_kb - 1),
                        skip_group_check=True)
            # drain PV psum to SBUF with plain copies (frees the pv banks
            # fast, keeps the exp pipeline pure); normalize happens in a
            # separate pass after the qg loop so the cross-engine epilogue
            # chain never interrupts the ACT/DVE exp streams.
            augA = paug.tile([128, 4 * 65], FP32, tag="aug")
            nc.vector.tensor_copy(augA[:], opsA[:])
            augB = paug.tile([128, 4 * 65], FP32, tag="aug")
            nc.vector.tensor_copy(augB[:], opsB[:])
            augs.append((qg, 0, augA))
            augs.append((qg, 1, augB))

        # ---------------- Phase C: normalize + store ----------------
        if "n" in phases:  # timing probe: skip normalize, raw aug DMA
            for qg, b, aug in augs:
                base = b * s + qg * QG
                nc.sync.dma_start(
                    out_d[base:base + QG, 0:64]
                    .rearrange("(c p) m -> p c m", p=128),
                    aug[:].rearrange("p (c e) -> p c e", e=65)[:, :, 0:64])
            return
        for qg, b, aug in augs:
            r3 = aug[:].rearrange("p (c e) -> p c e", e=65)
            rcp = prcp.tile([128, 4], FP32, tag="rcp")
            nc.vector.reciprocal(
                rcp[:].rearrange("p (c e) -> p c e", e=1),
                r3[:, :, 64:65])
            ofin = pout.tile([128, 4 * DK], FP32, tag="ofin")
            for c in range(4):
                nc.vector.tensor_scalar(
                    out=ofin[:, c * DK:(c + 1) * DK],
                    in0=r3[:, c, 0:DK], scalar1=rcp[:, c:c + 1],
                    scalar2=None, op0=mybir.AluOpType.mult)
            base = b * s + qg * QG
            nc.sync.dma_start(
                out_d[base:base + QG, :]
                .rearrange("(c p) m -> p c m", p=128),
                ofin[:].rearrange("p (c m) -> p c m", m=DK))


_NC_CACHE = {}


def _get_nc(s=S, reps=1, phases="AB"):
    key = (s, reps, phases)
    if key not in _NC_CACHE:
        _NC_CACHE[key] = build_nc(s, reps, phases)
    return _NC_CACHE[key]


def make_in_maps(inputs, s=S):
    x = np.ascontiguousarray(np.asarray(inputs["x"], dtype=np.float32))
    toks = B * s
    xbf = x.reshape(toks, D).astype(ml_dtypes.bfloat16)
    xbf = np.ascontiguousarray(xbf.reshape(toks, 4, 128).transpose(1, 0, 2))
    Wq = np.asarray(inputs["Wq"], dtype=np.float32)
    Wk = np.asarray(inputs["Wk"], dtype=np.float32)
    Wv = np.asarray(inputs["Wv"], dtype=np.float32)
    bq = np.asarray(inputs["bq"], dtype=np.float32)
    bk = np.asarray(inputs["bk"], dtype=np.float32)
    bv = np.asarray(inputs["bv"], dtype=np.float32)
    in_maps = []
    for h in range(N_CORES):
        wqk = np.concatenate([Wq[h], Wk[h]], axis=1).astype(ml_dtypes.bfloat16)
        in_maps.append({
            "xbf": xbf,
            "wqk": np.ascontiguousarray(wqk),
            "wv": np.ascontiguousarray(Wv[h].astype(ml_dtypes.bfloat16)),
            "bq": np.ascontiguousarray(bq[h].reshape(DK, 1)),
            "bk": np.ascontiguousarray(bk[h].reshape(DK, 1)),
            "bv4": np.ascontiguousarray(np.tile(bv[h], (128, 4))),
        })
    return in_maps


def assemble(results, s=S):
    toks = B * s
    out = np.empty((toks, HEADS * DK), dtype=np.float32)
    for h in range(N_CORES):
        out[:, h * DK:(h + 1) * DK] = results[h]["out"]
    return out.reshape(B, s, HEADS * DK)


def kernel(**inputs):
    nc = _get_nc(S)
    res = run_bass_kernel_spmd(nc, make_in_maps(inputs, S),
                               core_ids=list(range(N_CORES)))
    return assemble(res.results, S)
